# revision 27
# baseline (speedup 1.0000x reference)
"""Trainium2 Bass kernel for nn_DecoderRNN (GRU decoder with teacher forcing).

Strategy (8 NeuronCores, tensor-parallel over the vocab dim V):
  - Host: token prep (BOS + shifted target), embedding gather + ReLU,
    weight transposes/casts to fp16 device layouts.
  - Device (SPMD, identical program; per-core w_out/b_out shard differs):
      gxT = w_ih @ x^T precompute (PE, fp16)
      10 sequential GRU steps (PE matmuls + DVE/ACT gates, fp32 state),
      software-pipelined with the output projection: tile t's projection
      matmuls / +b_out / exp+row-accumulate / AllReduce(sum exp) overlap
      steps t+1, t+2. ln(S) is evaluated as a polynomial on DVE (bit-trick
      exponent + atanh series) so the ACT engine stays on one table set
      (tanh/exp) with zero mid-kernel table reloads.
      logp = logits - lse streamed out as fp32.
  - Host: gather shards -> full [B, T, V] log-softmax output + h_last.
"""

import os

import numpy as np
import ml_dtypes

import concourse.bass as bass
import concourse.mybir as mybir
import concourse.tile as tile
from concourse import bacc
from concourse.bass_utils import run_bass_kernel_spmd

F16NP = np.float16

P = 128          # partitions
B = 128          # batch
T = 10           # decode steps
H = 512          # hidden
KT = H // P      # 4 k-tiles over H
GJ = (3 * H) // P  # 12 gate-channel tiles
V = 50257
NCORES = 8
VS = 6656        # per-core vocab shard (13 * 512)
VPAD = VS * NCORES
NCH = VS // 512  # 13 chunks per shard
NGRP = 7         # 6x1024 + 1x512 psum groups
TB = T * B       # 1280
BOS = 0
NEG_BIG = -30000.0
LN2 = 0.6931471805599453

F32 = mybir.dt.float32
F16 = mybir.dt.float16
U32 = mybir.dt.uint32
ADD = mybir.AluOpType.add
MUL = mybir.AluOpType.mult
SUB = mybir.AluOpType.subtract
AF = mybir.ActivationFunctionType

TB_CHUNKS = [(0, 512), (512, 512), (1024, 256)]


def _tb_chunk_of_step(t):
    if t < 4:
        return 0, t * P
    if t < 8:
        return 1, (t - 4) * P
    return 2, (t - 8) * P


def _build_bass():
    nc = bacc.Bacc(
        "TRN2",
        target_bir_lowering=False,
        debug=False,
        enable_asserts=False,
        num_devices=NCORES,
    )

    # ---- kernel I/O ----
    xT = nc.dram_tensor("xT", [P, KT, TB], F16, kind="ExternalInput").ap()
    h0T = nc.dram_tensor("h0T", [P, KT, B], F32, kind="ExternalInput").ap()
    wih = nc.dram_tensor("wih", [P, KT, GJ, P], F16, kind="ExternalInput").ap()
    whh = nc.dram_tensor("whh", [P, KT, GJ, P], F16, kind="ExternalInput").ap()
    bgx = nc.dram_tensor("bgx", [P, GJ], F32, kind="ExternalInput").ap()
    bhn = nc.dram_tensor("bhn", [1, KT * P], F16, kind="ExternalInput").ap()
    wout = nc.dram_tensor("wout", [P, KT, VS], F16, kind="ExternalInput").ap()
    bout = nc.dram_tensor("bout", [1, VS], F16, kind="ExternalInput").ap()
    iden = nc.dram_tensor("iden", [P, P], F16, kind="ExternalInput").ap()

    logp = nc.dram_tensor("logp", [T, B, VS], F32, kind="ExternalOutput").ap()
    hT_out = nc.dram_tensor("hT_out", [P, KT, B], F32, kind="ExternalOutput").ap()

    with tile.TileContext(nc) as tc:
        from contextlib import ExitStack

        with ExitStack() as ctx:
            consts = ctx.enter_context(tc.tile_pool(name="consts", bufs=1))
            xsp = ctx.enter_context(tc.tile_pool(name="xsp", bufs=4))
            gxp = ctx.enter_context(tc.tile_pool(name="gxp", bufs=1))
            houts = ctx.enter_context(tc.tile_pool(name="houts", bufs=1))
            hsp = ctx.enter_context(tc.tile_pool(name="hsp", bufs=2))
            gtmp = ctx.enter_context(tc.tile_pool(name="gtmp", bufs=3))
            ltp = ctx.enter_context(tc.tile_pool(name="ltp", bufs=4))
            expp = ctx.enter_context(tc.tile_pool(name="expp", bufs=1))
            sredp = ctx.enter_context(tc.tile_pool(name="sredp", bufs=2))
            stp = ctx.enter_context(tc.tile_pool(name="stp", bufs=4))
            ps_gru = ctx.enter_context(tc.tile_pool(name="ps_gru", bufs=1, space="PSUM"))
            ps_mm = ctx.enter_context(tc.tile_pool(name="ps_mm", bufs=2, space="PSUM"))
            dramp = ctx.enter_context(tc.tile_pool(name="dramp", bufs=2, space="DRAM"))

            # ---- load constants (in usage order; big w_out DMAs go last) ----
            # w_ih borrows a logits-ring slot (dead after the gx phase)
            wih_sb = ltp.tile([P, KT, GJ, P], F16, name="wih_sb", tag="lt")
            nc.sync.dma_start(wih_sb[:], wih[:])
            bgx_sb = consts.tile([P, GJ], F32, name="bgx_sb")
            nc.sync.dma_start(bgx_sb[:], bgx[:])
            h0T_sb = consts.tile([P, KT, B], F32, name="h0T_sb")
            nc.sync.dma_start(h0T_sb[:], h0T[:])
            h0b = consts.tile([P, KT, B], F16, name="h0b")
            nc.vector.tensor_copy(h0b[:], h0T_sb[:])
            whh_sb = consts.tile([P, KT, GJ, P], F16, name="whh_sb")
            nc.sync.dma_start(whh_sb[:], whh[:])
            bhn_sb = consts.tile([1, KT * P], F16, name="bhn_sb")
            nc.sync.dma_start(bhn_sb[:], bhn[:])
            ones_sb = consts.tile([1, P], F16, name="ones_sb")
            nc.vector.memset(ones_sb[:], 1.0)
            iden_sb = consts.tile([P, P], F16, name="iden_sb")
            nc.sync.dma_start(iden_sb[:], iden[:])

            # ---- gx precompute (xT streamed from DRAM per chunk) ----
            gx_t = []
            for c, (o, n) in enumerate(TB_CHUNKS):
                g = gxp.tile([P, GJ, n], F16, name=f"gx{c}", tag=f"gx{c}")
                gx_t.append(g)
            for c, (o, n) in enumerate(TB_CHUNKS):
                xk = []
                for k in range(KT):
                    xt_ = xsp.tile([P, 512], F16, name=f"xt{c}{k}", tag="xs")
                    nc.sync.dma_start(xt_[:, :n], xT[:, k, o:o + n])
                    xk.append(xt_)
                for j in range(GJ):
                    psg = ps_mm.tile([P, 1024], F32, name="psg", tag="mm")
                    for k in range(KT):
                        nc.tensor.matmul(
                            psg[:, :n],
                            wih_sb[:, k, j, :],
                            xk[k][:, :n],
                            start=(k == 0),
                            stop=(k == KT - 1),
                        )
                    # psum drain + per-channel bias on the scalar engine
                    nc.scalar.activation(
                        gx_t[c][:, j, :], psg[:, :n], AF.Identity,
                        bias=bgx_sb[:, j:j + 1])

            # big output-projection constants: DMA after the gx inputs
            wout_sb = consts.tile([P, KT, VS], F16, name="wout_sb")
            nc.sync.dma_start(wout_sb[:], wout[:])
            bout_sb = consts.tile([P, VS], F16, name="bout_sb")
            nc.sync.dma_start(bout_sb[:], bout.to_broadcast((P, VS)))

            h_bf = [houts.tile([P, KT, B], F16, name=f"hb{t}", tag=f"hb{t}")
                    for t in range(T)]

            state = {"hf": h0T_sb, "hb": h0b}
            lt_tiles = [None] * T
            sp_tiles = [None] * T
            sg_tiles = [None] * T

            def gru_step(t):
                hprev_f, hprev_b = state["hf"], state["hb"]
                c_t, o_t = _tb_chunk_of_step(t)
                gxs = gx_t[c_t]

                def gx_slice(j0, j1):
                    return gxs[:, j0:j1, o_t:o_t + P]

                ps = ps_gru.tile([P, GJ, B], F32, name="psgru", tag="gru")
                for j in range(GJ):
                    for k in range(KT):
                        nc.tensor.matmul(
                            ps[:, j, :],
                            whh_sb[:, k, j, :],
                            hprev_b[:, k, :],
                            start=(k == 0),
                            stop=(k == KT - 1 and j < 8),
                        )
                    if j >= 8:
                        jj = j - 8
                        nc.tensor.matmul(
                            ps[:, j, :],
                            bhn_sb[:, jj * P:(jj + 1) * P],
                            ones_sb[:],
                            start=False,
                            stop=True,
                        )

                # tanh-only gates: sigmoid(x) = (1 + tanh(x/2)) / 2
                ta = gtmp.tile([P, 4, B], F32, name="ta", tag="gab")
                nc.vector.tensor_tensor(ta[:], ps[:, 0:4, :], gx_slice(0, 4), ADD)
                thr = gtmp.tile([P, 4, B], F32, name="thr", tag="gab")
                nc.scalar.activation(thr[:], ta[:], AF.Tanh, scale=0.5)
                tb_ = gtmp.tile([P, 4, B], F32, name="tb_", tag="gab")
                nc.vector.tensor_tensor(tb_[:], ps[:, 4:8, :], gx_slice(4, 8), ADD)
                thz = gtmp.tile([P, 4, B], F32, name="thz", tag="gab")
                nc.scalar.activation(thz[:], tb_[:], AF.Tanh, scale=0.5)
                # n = tanh(gx_n + r*gh_n), r = (thr+1)/2
                tc1 = gtmp.tile([P, 4, B], F32, name="tc1", tag="gcd")
                nc.vector.scalar_tensor_tensor(
                    tc1[:], thr[:], 1.0, ps[:, 8:12, :], ADD, MUL)
                tc2 = gtmp.tile([P, 4, B], F32, name="tc2", tag="gcd")
                nc.vector.scalar_tensor_tensor(
                    tc2[:], tc1[:], 0.5, gx_slice(8, 12), MUL, ADD)
                nn_ = gtmp.tile([P, 4, B], F32, name="nn_", tag="gcd")
                nc.scalar.activation(nn_[:], tc2[:], AF.Tanh)
                # h_new = n + z*(h_prev - n), z = (thz+1)/2
                td = gtmp.tile([P, 4, B], F32, name="td", tag="gcd")
                nc.vector.tensor_tensor(td[:], hprev_f[:], nn_[:], SUB)
                te = gtmp.tile([P, 4, B], F32, name="te", tag="gcd")
                nc.vector.scalar_tensor_tensor(te[:], thz[:], 1.0, td[:], ADD, MUL)
                hn = hsp.tile([P, KT, B], F32, name="hn", tag="h")
                nc.vector.scalar_tensor_tensor(hn[:], te[:], 0.5, nn_[:], MUL, ADD)
                nc.vector.tensor_copy(h_bf[t][:], hn[:])
                state["hf"], state["hb"] = hn, h_bf[t]

            def logits_mm_bias(t):
                lt = ltp.tile([P, VS], F16, name="lt", tag="lt")
                lt_tiles[t] = lt
                for gi in range(NGRP):
                    o = gi * 1024
                    n = min(1024, VS - o)
                    psl = ps_mm.tile([P, 1024], F32, name="psl", tag="mm")
                    for half in range(0, n, 512):
                        for k in range(KT):
                            nc.tensor.matmul(
                                psl[:, half:half + 512],
                                h_bf[t][:, k, :],
                                wout_sb[:, k, o + half:o + half + 512],
                                start=(k == 0),
                                stop=(k == KT - 1),
                            )
                    nc.vector.tensor_tensor(
                        lt[:, o:o + n], psl[:, :n], bout_sb[:, o:o + n], ADD)

            rg = [list(range(NCORES))]
            # AllReduce groups: pairs early, singles at the tail so the last
            # tiles don't wait on each other
            GROUPS = [[0, 1], [2, 3], [4, 5], [6, 7], [8], [9]]
            grp_of = {}
            for gidx, g in enumerate(GROUPS):
                for tt_ in g:
                    grp_of[tt_] = gidx
            sgrp_tiles = [None] * len(GROUPS)
            sg_tiles = [None] * len(GROUPS)
            EXPCH = [(0, 2048), (2048, 2048), (4096, 2048), (6144, 512)]

            def exps(t):
                lt = lt_tiles[t]
                gidx = grp_of[t]
                g = GROUPS[gidx]
                i = g.index(t)
                if i == 0:
                    sgrp_tiles[gidx] = sredp.tile(
                        [P, len(g)], F32, name="spr", tag="spr")
                sp = sredp.tile([P, 4], F32, name="sp", tag="sp")
                for gi, (o, n) in enumerate(EXPCH):
                    ex = expp.tile([P, 2048], F16, name="ex", tag="ex")
                    nc.scalar.activation(
                        ex[:, :n], lt[:, o:o + n], AF.Exp,
                        accum_out=sp[:, gi:gi + 1])
                nc.vector.tensor_reduce(
                    sgrp_tiles[gidx][:, i:i + 1], sp[:],
                    axis=mybir.AxisListType.X, op=ADD)

            def allreduce_group(gidx):
                ng = len(GROUPS[gidx])
                cc_in = dramp.tile([P, ng], F32, name="cc_in", tag=f"cci{ng}")
                cc_out = dramp.tile([P, ng], F32, name="cc_out", tag=f"cco{ng}",
                                    addr_space="Shared")
                nc.sync.dma_start(cc_in[:], sgrp_tiles[gidx][:])
                nc.gpsimd.collective_compute(
                    "AllReduce", ADD, replica_groups=rg,
                    ins=[cc_in.opt()], outs=[cc_out.opt()])
                s_glob = sredp.tile([P, 2], F32, name="s_glob", tag="sg")
                nc.sync.dma_start(s_glob[:, :ng], cc_out[:])
                sg_tiles[gidx] = s_glob

            def ptile(tag, dt=F32):
                return sredp.tile([P, 2], dt, name=tag, tag=tag)

            def lse_subtract_group(gidx):
                S = sg_tiles[gidx]
                Su = S.bitcast(U32)
                # lse = ln(S) via exponent bit-trick + atanh series on DVE
                ebu = ptile("ebu", U32)
                nc.vector.tensor_scalar(
                    out=ebu[:], in0=Su, scalar1=23, scalar2=None,
                    op0=mybir.AluOpType.logical_shift_right)
                ebf = ptile("ebf")
                nc.vector.tensor_copy(ebf[:], ebu[:])
                mu = ptile("mu", U32)
                nc.vector.tensor_scalar(
                    out=mu[:], in0=Su, scalar1=0x007FFFFF, scalar2=0x3F800000,
                    op0=mybir.AluOpType.bitwise_and,
                    op1=mybir.AluOpType.bitwise_or)
                m = mu.bitcast(F32)
                am = ptile("am")
                nc.vector.tensor_scalar(out=am[:], in0=m, scalar1=-1.0,
                                        scalar2=None, op0=ADD)
                bm = ptile("bm")
                nc.vector.tensor_scalar(out=bm[:], in0=m, scalar1=1.0,
                                        scalar2=None, op0=ADD)
                rb = ptile("rb")
                nc.vector.reciprocal(rb[:], bm[:])
                z = ptile("z")
                nc.vector.tensor_tensor(z[:], am[:], rb[:], MUL)
                z2 = ptile("z2")
                nc.vector.tensor_tensor(z2[:], z[:], z[:], MUL)
                # p = ((1/9*z2 + 1/7)*z2 + 1/5)*z2 + 1/3)*z2 + 1
                p = ptile("p0")
                nc.vector.tensor_scalar(out=p[:], in0=z2[:], scalar1=1.0 / 9,
                                        scalar2=1.0 / 7, op0=MUL, op1=ADD)
                for cst, nm in ((1.0 / 5, "p1"), (1.0 / 3, "p2"), (1.0, "p3")):
                    pz = ptile(nm + "m")
                    nc.vector.tensor_tensor(pz[:], p[:], z2[:], MUL)
                    p2 = ptile(nm)
                    nc.vector.tensor_scalar(out=p2[:], in0=pz[:], scalar1=cst,
                                            scalar2=None, op0=ADD)
                    p = p2
                lnm = ptile("lnm")
                nc.vector.scalar_tensor_tensor(lnm[:], p[:], 2.0, z[:], MUL, MUL)
                ls0 = ptile("ls0")
                nc.vector.scalar_tensor_tensor(ls0[:], ebf[:], LN2, lnm[:], MUL, ADD)
                lse = ptile("lse")
                nc.vector.tensor_scalar(out=lse[:], in0=ls0[:],
                                        scalar1=-127.0 * LN2, scalar2=None, op0=ADD)
                nlse = ptile("nlse")
                nc.vector.tensor_scalar(out=nlse[:], in0=lse[:], scalar1=-1.0,
                                        scalar2=None, op0=MUL)
                for i, t in enumerate(GROUPS[gidx]):
                    lt = lt_tiles[t]
                    for ci in range(NCH):
                        o = ci * 512
                        st = stp.tile([P, 512], F32, name="st", tag="st")
                        if ci % 3 == 0:
                            nc.vector.tensor_scalar(
                                out=st[:], in0=lt[:, o:o + 512],
                                scalar1=lse[:, i:i + 1], scalar2=None, op0=SUB)
                        else:
                            nc.scalar.activation(
                                st[:], lt[:, o:o + 512], AF.Identity,
                                bias=nlse[:, i:i + 1])
                        nc.sync.dma_start(logp[t, :, o:o + 512], st[:])

            # ---- software-pipelined main loop ----
            # iteration t: GRU step t; projection matmuls of tile t-1 (these
            # hide step t's serial gate chain on PE); exp+accum of tile t-2;
            # AllReduce once a group of tiles has summed; lse+subtract a
            # group one iteration after its AllReduce was issued (emitted
            # last so gate ops stay at the front of the in-order queues).
            ar_iter = {}
            for gidx, g in enumerate(GROUPS):
                ar_iter[gidx] = g[-1] + 2  # iteration where AR is emitted
            for t in range(T + 4):
                if t < T:
                    gru_step(t)
                if 0 <= t - 1 < T:
                    logits_mm_bias(t - 1)
                v = t - 2
                if 0 <= v < T:
                    exps(v)
                    gidx = grp_of[v]
                    if v == GROUPS[gidx][-1]:
                        allreduce_group(gidx)
                for gidx in range(len(GROUPS)):
                    if ar_iter[gidx] == t - 1:
                        lse_subtract_group(gidx)

            nc.sync.dma_start(hT_out[:], state["hf"][:])

    nc.compile()
    return nc


_NC_CACHE = {}


def _get_bass():
    if "nc" not in _NC_CACHE:
        _NC_CACHE["nc"] = _build_bass()
    return _NC_CACHE["nc"]


LAST_RESULT = None


def kernel(input=None, hidden=None, target=None, emb=None, w_ih=None, w_hh=None,
           b_ih=None, b_hh=None, w_out=None, b_out=None):
    global LAST_RESULT
    target = np.asarray(target)
    emb = np.asarray(emb, dtype=np.float32)
    hidden = np.asarray(hidden, dtype=np.float32)
    w_ih_ = np.asarray(w_ih, dtype=np.float32)
    w_hh_ = np.asarray(w_hh, dtype=np.float32)
    b_ih_ = np.asarray(b_ih, dtype=np.float32)
    b_hh_ = np.asarray(b_hh, dtype=np.float32)
    w_out_ = np.asarray(w_out, dtype=np.float32)
    b_out_ = np.asarray(b_out, dtype=np.float32)

    # ---- host prep ----
    tokens = np.concatenate(
        [np.full((B, 1), BOS, dtype=np.int64), target[:, : T - 1].astype(np.int64)],
        axis=1,
    )  # [B, T]
    x = emb[tokens]  # [B, T, H]
    np.maximum(x, 0.0, out=x)
    xT = np.ascontiguousarray(
        x.reshape(B, T, KT, P).transpose(3, 2, 1, 0)
    ).astype(F16NP).reshape(P, KT, TB)

    h0 = hidden[0]  # [B, H]
    h0T = np.ascontiguousarray(h0.reshape(B, KT, P).transpose(2, 1, 0))

    wihT = np.ascontiguousarray(
        w_ih_.reshape(GJ, P, KT, P).transpose(3, 2, 0, 1)
    ).astype(F16NP)
    whhT = np.ascontiguousarray(
        w_hh_.reshape(GJ, P, KT, P).transpose(3, 2, 0, 1)
    ).astype(F16NP)

    bgx = b_ih_.copy()
    bgx[: 2 * H] += b_hh_[: 2 * H]  # fold b_hh for r,z; n keeps b_ih only
    bgx_dev = np.ascontiguousarray(bgx.reshape(GJ, P).T)
    bhn_dev = np.ascontiguousarray(b_hh_[2 * H:].reshape(1, H)).astype(F16NP)

    wpad = np.zeros((VPAD, H), dtype=np.float32)
    wpad[:V] = w_out_
    woutT_all = np.ascontiguousarray(
        wpad.reshape(VPAD, KT, P).transpose(2, 1, 0)
    ).astype(F16NP)  # [P, KT, VPAD]
    bpad = np.full((VPAD,), NEG_BIG, dtype=np.float32)
    bpad[:V] = b_out_
    bpad_f16 = bpad.astype(F16NP).reshape(1, VPAD)

    nc = _get_bass()
    iden_np = np.eye(P, dtype=F16NP)
    in_maps = []
    for c in range(NCORES):
        v0 = c * VS
        in_maps.append({
            "xT": xT,
            "h0T": h0T,
            "wih": wihT,
            "whh": whhT,
            "bgx": bgx_dev,
            "bhn": bhn_dev,
            "wout": np.ascontiguousarray(woutT_all[:, :, v0:v0 + VS]),
            "bout": np.ascontiguousarray(bpad_f16[:, v0:v0 + VS]),
            "iden": iden_np,
        })

    trace = bool(int(os.environ.get("KERNEL_TRACE", "0")))
    res = run_bass_kernel_spmd(
        nc, in_maps, core_ids=list(range(NCORES)), trace=trace,
    )
    LAST_RESULT = res

    # ---- gather ----
    logp_full = np.empty((B, T, V), dtype=np.float32)
    for c in range(NCORES):
        lp_c = res.results[c]["logp"]  # [T, B, VS]
        v0 = c * VS
        v1 = min(V, v0 + VS)
        if v1 > v0:
            logp_full[:, :, v0:v1] = lp_c.transpose(1, 0, 2)[:, :, : v1 - v0]
    hT = res.results[0]["hT_out"]  # [p, k, b]
    h_last = np.ascontiguousarray(hT.transpose(2, 1, 0).reshape(1, B, H))
    return logp_full, h_last


# revision 28
# speedup vs baseline: 1.0553x; 1.0553x over previous
"""Trainium2 Bass kernel for nn_DecoderRNN (GRU decoder with teacher forcing).

Strategy (8 NeuronCores, tensor-parallel over the vocab dim V):
  - Host: token prep (BOS + shifted target), embedding gather + ReLU,
    weight transposes/casts to fp16 device layouts.
  - Device (SPMD, identical program; per-core w_out/b_out shard differs):
      gxT = w_ih @ x^T precompute (PE, fp16)
      10 sequential GRU steps (PE matmuls + DVE/ACT gates, fp32 state),
      software-pipelined with the output projection: tile t's projection
      matmuls / +b_out / exp+row-accumulate / AllReduce(sum exp) overlap
      steps t+1, t+2. ln(S) is evaluated as a polynomial on DVE (bit-trick
      exponent + atanh series) so the ACT engine stays on one table set
      (tanh/exp) with zero mid-kernel table reloads.
      logp = logits - lse streamed out as fp32.
  - Host: gather shards -> full [B, T, V] log-softmax output + h_last.
"""

import os

import numpy as np
import ml_dtypes

import concourse.bass as bass
import concourse.mybir as mybir
import concourse.tile as tile
from concourse import bacc
from concourse.bass_utils import run_bass_kernel_spmd

F16NP = np.float16

P = 128          # partitions
B = 128          # batch
T = 10           # decode steps
H = 512          # hidden
KT = H // P      # 4 k-tiles over H
GJ = (3 * H) // P  # 12 gate-channel tiles
V = 50257
NCORES = 8
VS = 6656        # per-core vocab shard (13 * 512)
VPAD = VS * NCORES
NCH = VS // 512  # 13 chunks per shard
NGRP = 7         # 6x1024 + 1x512 psum groups
TB = T * B       # 1280
BOS = 0
NEG_BIG = -30000.0
LN2 = 0.6931471805599453

F32 = mybir.dt.float32
F16 = mybir.dt.float16
U32 = mybir.dt.uint32
ADD = mybir.AluOpType.add
MUL = mybir.AluOpType.mult
SUB = mybir.AluOpType.subtract
AF = mybir.ActivationFunctionType

TB_CHUNKS = [(0, 512), (512, 512), (1024, 256)]


def _tb_chunk_of_step(t):
    if t < 4:
        return 0, t * P
    if t < 8:
        return 1, (t - 4) * P
    return 2, (t - 8) * P


def _build_bass():
    nc = bacc.Bacc(
        "TRN2",
        target_bir_lowering=False,
        debug=False,
        enable_asserts=False,
        num_devices=NCORES,
    )

    # ---- kernel I/O ----
    xT = nc.dram_tensor("xT", [P, KT, TB], F16, kind="ExternalInput").ap()
    h0T = nc.dram_tensor("h0T", [P, KT, B], F32, kind="ExternalInput").ap()
    wih = nc.dram_tensor("wih", [P, KT, GJ, P], F16, kind="ExternalInput").ap()
    whh = nc.dram_tensor("whh", [P, KT, GJ, P], F16, kind="ExternalInput").ap()
    bgx = nc.dram_tensor("bgx", [P, GJ], F32, kind="ExternalInput").ap()
    bhn = nc.dram_tensor("bhn", [1, KT * P], F16, kind="ExternalInput").ap()
    wout = nc.dram_tensor("wout", [P, KT, VS], F16, kind="ExternalInput").ap()
    bout = nc.dram_tensor("bout", [1, VS], F16, kind="ExternalInput").ap()
    iden = nc.dram_tensor("iden", [P, P], F16, kind="ExternalInput").ap()

    logp = nc.dram_tensor("logp", [T, B, VS], F32, kind="ExternalOutput").ap()
    hT_out = nc.dram_tensor("hT_out", [P, KT, B], F32, kind="ExternalOutput").ap()

    with tile.TileContext(nc) as tc:
        from contextlib import ExitStack

        with ExitStack() as ctx:
            consts = ctx.enter_context(tc.tile_pool(name="consts", bufs=1))
            xsp = ctx.enter_context(tc.tile_pool(name="xsp", bufs=4))
            gxp = ctx.enter_context(tc.tile_pool(name="gxp", bufs=1))
            houts = ctx.enter_context(tc.tile_pool(name="houts", bufs=1))
            hsp = ctx.enter_context(tc.tile_pool(name="hsp", bufs=2))
            gtmp = ctx.enter_context(tc.tile_pool(name="gtmp", bufs=3))
            ltp = ctx.enter_context(tc.tile_pool(name="ltp", bufs=4))
            expp = ctx.enter_context(tc.tile_pool(name="expp", bufs=1))
            sredp = ctx.enter_context(tc.tile_pool(name="sredp", bufs=2))
            stp = ctx.enter_context(tc.tile_pool(name="stp", bufs=4))
            ps_gru = ctx.enter_context(tc.tile_pool(name="ps_gru", bufs=1, space="PSUM"))
            ps_mm = ctx.enter_context(tc.tile_pool(name="ps_mm", bufs=2, space="PSUM"))
            dramp = ctx.enter_context(tc.tile_pool(name="dramp", bufs=2, space="DRAM"))

            # ---- load constants (in usage order; big w_out DMAs go last) ----
            # w_ih borrows a logits-ring slot (dead after the gx phase)
            wih_sb = ltp.tile([P, KT, GJ, P], F16, name="wih_sb", tag="lt")
            nc.sync.dma_start(wih_sb[:], wih[:])
            bgx_sb = consts.tile([P, GJ], F32, name="bgx_sb")
            nc.sync.dma_start(bgx_sb[:], bgx[:])
            h0T_sb = consts.tile([P, KT, B], F32, name="h0T_sb")
            nc.sync.dma_start(h0T_sb[:], h0T[:])
            h0b = consts.tile([P, KT, B], F16, name="h0b")
            nc.vector.tensor_copy(h0b[:], h0T_sb[:])
            whh_sb = consts.tile([P, KT, GJ, P], F16, name="whh_sb")
            nc.sync.dma_start(whh_sb[:], whh[:])
            bhn_sb = consts.tile([1, KT * P], F16, name="bhn_sb")
            nc.sync.dma_start(bhn_sb[:], bhn[:])
            ones_sb = consts.tile([1, P], F16, name="ones_sb")
            nc.vector.memset(ones_sb[:], 1.0)
            iden_sb = consts.tile([P, P], F16, name="iden_sb")
            nc.sync.dma_start(iden_sb[:], iden[:])

            # ---- gx precompute (xT streamed from DRAM per chunk) ----
            gx_t = []
            for c, (o, n) in enumerate(TB_CHUNKS):
                g = gxp.tile([P, GJ, n], F16, name=f"gx{c}", tag=f"gx{c}")
                gx_t.append(g)
            for c, (o, n) in enumerate(TB_CHUNKS):
                xk = []
                for k in range(KT):
                    xt_ = xsp.tile([P, 512], F16, name=f"xt{c}{k}", tag="xs")
                    nc.sync.dma_start(xt_[:, :n], xT[:, k, o:o + n])
                    xk.append(xt_)
                for j in range(GJ):
                    psg = ps_mm.tile([P, 1024], F32, name="psg", tag="mm")
                    for k in range(KT):
                        nc.tensor.matmul(
                            psg[:, :n],
                            wih_sb[:, k, j, :],
                            xk[k][:, :n],
                            start=(k == 0),
                            stop=(k == KT - 1),
                        )
                    # psum drain + per-channel bias on the scalar engine
                    nc.scalar.activation(
                        gx_t[c][:, j, :], psg[:, :n], AF.Identity,
                        bias=bgx_sb[:, j:j + 1])

            # big output-projection constants: DMA after the gx inputs
            wout_sb = consts.tile([P, KT, VS], F16, name="wout_sb")
            nc.sync.dma_start(wout_sb[:], wout[:])
            bout_sb = consts.tile([P, VS], F16, name="bout_sb")
            nc.sync.dma_start(bout_sb[:], bout.to_broadcast((P, VS)))

            h_bf = [houts.tile([P, KT, B], F16, name=f"hb{t}", tag=f"hb{t}")
                    for t in range(T)]

            state = {"hf": h0T_sb, "hb": h0b}
            lt_tiles = [None] * T
            sp_tiles = [None] * T
            sg_tiles = [None] * T

            def gru_step(t):
                hprev_f, hprev_b = state["hf"], state["hb"]
                c_t, o_t = _tb_chunk_of_step(t)
                gxs = gx_t[c_t]

                def gx_slice(j0, j1):
                    return gxs[:, j0:j1, o_t:o_t + P]

                ps = ps_gru.tile([P, GJ, B], F32, name="psgru", tag="gru")
                for j in range(GJ):
                    for k in range(KT):
                        nc.tensor.matmul(
                            ps[:, j, :],
                            whh_sb[:, k, j, :],
                            hprev_b[:, k, :],
                            start=(k == 0),
                            stop=(k == KT - 1 and j < 8),
                        )
                    if j >= 8:
                        jj = j - 8
                        nc.tensor.matmul(
                            ps[:, j, :],
                            bhn_sb[:, jj * P:(jj + 1) * P],
                            ones_sb[:],
                            start=False,
                            stop=True,
                        )

                # tanh-only gates: sigmoid(x) = (1 + tanh(x/2)) / 2
                ta = gtmp.tile([P, 4, B], F32, name="ta", tag="gab")
                nc.vector.tensor_tensor(ta[:], ps[:, 0:4, :], gx_slice(0, 4), ADD)
                thr = gtmp.tile([P, 4, B], F32, name="thr", tag="gab")
                nc.scalar.activation(thr[:], ta[:], AF.Tanh, scale=0.5)
                tb_ = gtmp.tile([P, 4, B], F32, name="tb_", tag="gab")
                nc.vector.tensor_tensor(tb_[:], ps[:, 4:8, :], gx_slice(4, 8), ADD)
                thz = gtmp.tile([P, 4, B], F32, name="thz", tag="gab")
                nc.scalar.activation(thz[:], tb_[:], AF.Tanh, scale=0.5)
                # n = tanh(gx_n + r*gh_n), r = (thr+1)/2
                tc1 = gtmp.tile([P, 4, B], F32, name="tc1", tag="gcd")
                nc.vector.scalar_tensor_tensor(
                    tc1[:], thr[:], 1.0, ps[:, 8:12, :], ADD, MUL)
                tc2 = gtmp.tile([P, 4, B], F32, name="tc2", tag="gcd")
                nc.vector.scalar_tensor_tensor(
                    tc2[:], tc1[:], 0.5, gx_slice(8, 12), MUL, ADD)
                nn_ = gtmp.tile([P, 4, B], F32, name="nn_", tag="gcd")
                nc.scalar.activation(nn_[:], tc2[:], AF.Tanh)
                # h_new = n + z*(h_prev - n), z = (thz+1)/2
                td = gtmp.tile([P, 4, B], F32, name="td", tag="gcd")
                nc.vector.tensor_tensor(td[:], hprev_f[:], nn_[:], SUB)
                te = gtmp.tile([P, 4, B], F32, name="te", tag="gcd")
                nc.vector.scalar_tensor_tensor(te[:], thz[:], 1.0, td[:], ADD, MUL)
                hn = hsp.tile([P, KT, B], F32, name="hn", tag="h")
                nc.vector.scalar_tensor_tensor(hn[:], te[:], 0.5, nn_[:], MUL, ADD)
                nc.vector.tensor_copy(h_bf[t][:], hn[:])
                state["hf"], state["hb"] = hn, h_bf[t]

            def logits_mm_bias(t):
                lt = ltp.tile([P, VS], F16, name="lt", tag="lt")
                lt_tiles[t] = lt
                for gi in range(NGRP):
                    o = gi * 1024
                    n = min(1024, VS - o)
                    psl = ps_mm.tile([P, 1024], F32, name="psl", tag="mm")
                    for half in range(0, n, 512):
                        for k in range(KT):
                            nc.tensor.matmul(
                                psl[:, half:half + 512],
                                h_bf[t][:, k, :],
                                wout_sb[:, k, o + half:o + half + 512],
                                start=(k == 0),
                                stop=(k == KT - 1),
                            )
                    nc.vector.tensor_tensor(
                        lt[:, o:o + n], psl[:, :n], bout_sb[:, o:o + n], ADD)

            rg = [list(range(NCORES))]
            # AllReduce groups: pairs early, singles at the tail so the last
            # tiles don't wait on each other
            GROUPS = [[0, 1], [2, 3], [4, 5], [6, 7], [8, 9]]
            grp_of = {}
            for gidx, g in enumerate(GROUPS):
                for tt_ in g:
                    grp_of[tt_] = gidx
            sgrp_tiles = [None] * len(GROUPS)
            sg_tiles = [None] * len(GROUPS)
            EXPCH = [(0, 2048), (2048, 2048), (4096, 2048), (6144, 512)]

            def exps(t):
                lt = lt_tiles[t]
                gidx = grp_of[t]
                g = GROUPS[gidx]
                i = g.index(t)
                if i == 0:
                    sgrp_tiles[gidx] = sredp.tile(
                        [P, len(g)], F32, name="spr", tag="spr")
                sp = sredp.tile([P, 4], F32, name="sp", tag="sp")
                for gi, (o, n) in enumerate(EXPCH):
                    ex = expp.tile([P, 2048], F16, name="ex", tag="ex")
                    nc.scalar.activation(
                        ex[:, :n], lt[:, o:o + n], AF.Exp,
                        accum_out=sp[:, gi:gi + 1])
                nc.vector.tensor_reduce(
                    sgrp_tiles[gidx][:, i:i + 1], sp[:],
                    axis=mybir.AxisListType.X, op=ADD)

            def allreduce_group(gidx):
                ng = len(GROUPS[gidx])
                cc_in = dramp.tile([P, ng], F32, name="cc_in", tag=f"cci{ng}")
                cc_out = dramp.tile([P, ng], F32, name="cc_out", tag=f"cco{ng}",
                                    addr_space="Shared")
                nc.sync.dma_start(cc_in[:], sgrp_tiles[gidx][:])
                nc.gpsimd.collective_compute(
                    "AllReduce", ADD, replica_groups=rg,
                    ins=[cc_in.opt()], outs=[cc_out.opt()])
                s_glob = sredp.tile([P, 2], F32, name="s_glob", tag="sg")
                nc.sync.dma_start(s_glob[:, :ng], cc_out[:])
                sg_tiles[gidx] = s_glob

            def ptile(tag, dt=F32):
                return sredp.tile([P, 2], dt, name=tag, tag=tag)

            def lse_subtract_group(gidx):
                S = sg_tiles[gidx]
                Su = S.bitcast(U32)
                # lse = ln(S) via exponent bit-trick + atanh series on DVE
                ebu = ptile("ebu", U32)
                nc.vector.tensor_scalar(
                    out=ebu[:], in0=Su, scalar1=23, scalar2=None,
                    op0=mybir.AluOpType.logical_shift_right)
                ebf = ptile("ebf")
                nc.vector.tensor_copy(ebf[:], ebu[:])
                mu = ptile("mu", U32)
                nc.vector.tensor_scalar(
                    out=mu[:], in0=Su, scalar1=0x007FFFFF, scalar2=0x3F800000,
                    op0=mybir.AluOpType.bitwise_and,
                    op1=mybir.AluOpType.bitwise_or)
                m = mu.bitcast(F32)
                am = ptile("am")
                nc.vector.tensor_scalar(out=am[:], in0=m, scalar1=-1.0,
                                        scalar2=None, op0=ADD)
                bm = ptile("bm")
                nc.vector.tensor_scalar(out=bm[:], in0=m, scalar1=1.0,
                                        scalar2=None, op0=ADD)
                rb = ptile("rb")
                nc.vector.reciprocal(rb[:], bm[:])
                z = ptile("z")
                nc.vector.tensor_tensor(z[:], am[:], rb[:], MUL)
                z2 = ptile("z2")
                nc.vector.tensor_tensor(z2[:], z[:], z[:], MUL)
                # p = ((1/9*z2 + 1/7)*z2 + 1/5)*z2 + 1/3)*z2 + 1
                p = ptile("p0")
                nc.vector.tensor_scalar(out=p[:], in0=z2[:], scalar1=1.0 / 9,
                                        scalar2=1.0 / 7, op0=MUL, op1=ADD)
                for cst, nm in ((1.0 / 5, "p1"), (1.0 / 3, "p2"), (1.0, "p3")):
                    pz = ptile(nm + "m")
                    nc.vector.tensor_tensor(pz[:], p[:], z2[:], MUL)
                    p2 = ptile(nm)
                    nc.vector.tensor_scalar(out=p2[:], in0=pz[:], scalar1=cst,
                                            scalar2=None, op0=ADD)
                    p = p2
                lnm = ptile("lnm")
                nc.vector.scalar_tensor_tensor(lnm[:], p[:], 2.0, z[:], MUL, MUL)
                ls0 = ptile("ls0")
                nc.vector.scalar_tensor_tensor(ls0[:], ebf[:], LN2, lnm[:], MUL, ADD)
                lse = ptile("lse")
                nc.vector.tensor_scalar(out=lse[:], in0=ls0[:],
                                        scalar1=-127.0 * LN2, scalar2=None, op0=ADD)
                nlse = ptile("nlse")
                nc.vector.tensor_scalar(out=nlse[:], in0=lse[:], scalar1=-1.0,
                                        scalar2=None, op0=MUL)
                for i, t in enumerate(GROUPS[gidx]):
                    lt = lt_tiles[t]
                    for ci in range(NCH):
                        o = ci * 512
                        st = stp.tile([P, 512], F32, name="st", tag="st")
                        if ci % 3 == 0:
                            nc.vector.tensor_scalar(
                                out=st[:], in0=lt[:, o:o + 512],
                                scalar1=lse[:, i:i + 1], scalar2=None, op0=SUB)
                        else:
                            nc.scalar.activation(
                                st[:], lt[:, o:o + 512], AF.Identity,
                                bias=nlse[:, i:i + 1])
                        nc.sync.dma_start(logp[t, :, o:o + 512], st[:])

            # ---- software-pipelined main loop ----
            # iteration t: GRU step t; projection matmuls of tile t-1 (these
            # hide step t's serial gate chain on PE); exp+accum of tile t-2;
            # AllReduce once a group of tiles has summed; lse+subtract a
            # group one iteration after its AllReduce was issued (emitted
            # last so gate ops stay at the front of the in-order queues).
            ar_iter = {}
            for gidx, g in enumerate(GROUPS):
                ar_iter[gidx] = g[-1] + 2  # iteration where AR is emitted
            for t in range(T + 4):
                if t < T:
                    gru_step(t)
                if 0 <= t - 1 < T:
                    logits_mm_bias(t - 1)
                v = t - 2
                if 0 <= v < T:
                    exps(v)
                    gidx = grp_of[v]
                    if v == GROUPS[gidx][-1]:
                        allreduce_group(gidx)
                for gidx in range(len(GROUPS)):
                    if ar_iter[gidx] == t - 1:
                        lse_subtract_group(gidx)

            nc.sync.dma_start(hT_out[:], state["hf"][:])

    nc.compile()
    return nc


_NC_CACHE = {}


def _get_bass():
    if "nc" not in _NC_CACHE:
        _NC_CACHE["nc"] = _build_bass()
    return _NC_CACHE["nc"]


LAST_RESULT = None


def kernel(input=None, hidden=None, target=None, emb=None, w_ih=None, w_hh=None,
           b_ih=None, b_hh=None, w_out=None, b_out=None):
    global LAST_RESULT
    target = np.asarray(target)
    emb = np.asarray(emb, dtype=np.float32)
    hidden = np.asarray(hidden, dtype=np.float32)
    w_ih_ = np.asarray(w_ih, dtype=np.float32)
    w_hh_ = np.asarray(w_hh, dtype=np.float32)
    b_ih_ = np.asarray(b_ih, dtype=np.float32)
    b_hh_ = np.asarray(b_hh, dtype=np.float32)
    w_out_ = np.asarray(w_out, dtype=np.float32)
    b_out_ = np.asarray(b_out, dtype=np.float32)

    # ---- host prep ----
    tokens = np.concatenate(
        [np.full((B, 1), BOS, dtype=np.int64), target[:, : T - 1].astype(np.int64)],
        axis=1,
    )  # [B, T]
    x = emb[tokens]  # [B, T, H]
    np.maximum(x, 0.0, out=x)
    xT = np.ascontiguousarray(
        x.reshape(B, T, KT, P).transpose(3, 2, 1, 0)
    ).astype(F16NP).reshape(P, KT, TB)

    h0 = hidden[0]  # [B, H]
    h0T = np.ascontiguousarray(h0.reshape(B, KT, P).transpose(2, 1, 0))

    wihT = np.ascontiguousarray(
        w_ih_.reshape(GJ, P, KT, P).transpose(3, 2, 0, 1)
    ).astype(F16NP)
    whhT = np.ascontiguousarray(
        w_hh_.reshape(GJ, P, KT, P).transpose(3, 2, 0, 1)
    ).astype(F16NP)

    bgx = b_ih_.copy()
    bgx[: 2 * H] += b_hh_[: 2 * H]  # fold b_hh for r,z; n keeps b_ih only
    bgx_dev = np.ascontiguousarray(bgx.reshape(GJ, P).T)
    bhn_dev = np.ascontiguousarray(b_hh_[2 * H:].reshape(1, H)).astype(F16NP)

    wpad = np.zeros((VPAD, H), dtype=np.float32)
    wpad[:V] = w_out_
    woutT_all = np.ascontiguousarray(
        wpad.reshape(VPAD, KT, P).transpose(2, 1, 0)
    ).astype(F16NP)  # [P, KT, VPAD]
    bpad = np.full((VPAD,), NEG_BIG, dtype=np.float32)
    bpad[:V] = b_out_
    bpad_f16 = bpad.astype(F16NP).reshape(1, VPAD)

    nc = _get_bass()
    iden_np = np.eye(P, dtype=F16NP)
    in_maps = []
    for c in range(NCORES):
        v0 = c * VS
        in_maps.append({
            "xT": xT,
            "h0T": h0T,
            "wih": wihT,
            "whh": whhT,
            "bgx": bgx_dev,
            "bhn": bhn_dev,
            "wout": np.ascontiguousarray(woutT_all[:, :, v0:v0 + VS]),
            "bout": np.ascontiguousarray(bpad_f16[:, v0:v0 + VS]),
            "iden": iden_np,
        })

    trace = bool(int(os.environ.get("KERNEL_TRACE", "0")))
    res = run_bass_kernel_spmd(
        nc, in_maps, core_ids=list(range(NCORES)), trace=trace,
    )
    LAST_RESULT = res

    # ---- gather ----
    logp_full = np.empty((B, T, V), dtype=np.float32)
    for c in range(NCORES):
        lp_c = res.results[c]["logp"]  # [T, B, VS]
        v0 = c * VS
        v1 = min(V, v0 + VS)
        if v1 > v0:
            logp_full[:, :, v0:v1] = lp_c.transpose(1, 0, 2)[:, :, : v1 - v0]
    hT = res.results[0]["hT_out"]  # [p, k, b]
    h_last = np.ascontiguousarray(hT.transpose(2, 1, 0).reshape(1, B, H))
    return logp_full, h_last
